# revision 46
# baseline (speedup 1.0000x reference)
"""Trainium2 8-core fused attention kernel (QKV proj + RMSNorm + RoPE + SDPA + out proj).

Sharding: tensor-parallel over heads. Each of the 8 cores computes 2 of the 16
heads end-to-end (QKV projection with its Wqkv column shard, per-head RMSNorm +
RoPE, full softmax attention), then AllToAlls redistribute the per-head
attention outputs so every core holds all 1024 attention channels for 1/8 of
the tokens and applies the full Wout to its token shard.

v3.2: jitter-free attention phases. All stage-A (both batches) runs upfront in
phase 1; the attention loops then carry nothing but scores/exp/AV, with exp
alternating whole-tile between ACT (even kc, exact) and DVE (odd kc,
Schraudolph) so both stay well under the PE's per-iteration budget and the PE
never drops out of its max p-state. Per-qt softmax drains are deferred into
the next q-tile's odd-kc slots (when ACT is exp-idle). The batch-1 AllToAll is
split in two (q-tiles 0-3 fire mid-phase) by remapping output chunk ownership,
shrinking the exposed tail.

Self-contained: hardcodes all shapes from the problem spec.
"""
import os
import sys
import types

import numpy as np
import ml_dtypes

sys.path.insert(0, "/opt/trn_rl_repo")

from concourse import bass, bacc, tile, mybir  # noqa: E402
from concourse.bass_utils import run_bass_kernel_spmd  # noqa: E402
from concourse.masks import make_identity  # noqa: E402

B, N, C, H, D = 2, 4096, 1024, 16, 64
NCORES = 8
TOK = B * N            # 8192 global tokens
NB = N // 128          # 32 token tiles per batch
NMACRO = N // 256      # 16 macro tiles (256 tok) per batch
QTILE = 512
NQT = N // QTILE       # 8 q tiles per batch
KC = N // 128          # 32 key chunks per batch
SHARD = TOK // NCORES  # 1024 tokens per core shard
EPS = 1e-6

F32 = mybir.dt.float32
BF16 = mybir.dt.bfloat16
U16 = mybir.dt.uint16
ALU = mybir.AluOpType
ACTF = mybir.ActivationFunctionType

# Schraudolph exp-via-bits for the DVE: bf16 bitpattern of exp(0.125*s) is
# approx round(A*s + B) as uint16 (error <= +-4.2%, rms 1.8%). Odd key-chunks
# use this (50% of elements); the bias cancels between softmax numerator and
# denominator.
SCHRAUD_A = 16.0 / np.log(2.0)
SCHRAUD_B = 16248.75

_CACHE = {}
_LAST_RESULT = None


def _install_profile_shim():
    """trn_boot skips the NTFF hook when antenv.axon_hooks is missing; supply it."""
    try:
        import antenv
        if getattr(antenv, "axon_hooks", None) is not None:
            return
        from trn_agent_boot.trn_boot import _ntff_profile_via_ctypes
        hook = _ntff_profile_via_ctypes("/opt/axon/libaxon_pjrt.so")
        if hook is None:
            return
        mod = types.ModuleType("antenv.axon_hooks")
        state = {"hook": hook}
        mod.get_axon_ntff_profile_hook = lambda: state["hook"]
        mod.set_axon_ntff_profile_hook = lambda h: state.__setitem__("hook", h)
        sys.modules["antenv.axon_hooks"] = mod
        antenv.axon_hooks = mod
    except Exception:
        pass


def _build_graph():
    nc = bacc.Bacc("TRN2", target_bir_lowering=False, debug=False,
                   enable_asserts=True, num_devices=NCORES)

    hsT_d = nc.dram_tensor("hsT", [C, TOK], BF16, kind="ExternalInput")
    wqkv_d = nc.dram_tensor("wqkv", [C, 384], BF16, kind="ExternalInput")
    trigc_d = nc.dram_tensor("trigc", [N, 256], BF16, kind="ExternalInput")
    trigs_d = nc.dram_tensor("trigs", [N, 256], BF16, kind="ExternalInput")
    wout_d = nc.dram_tensor("wout", [C, C], BF16, kind="ExternalInput")
    out_d = nc.dram_tensor("out", [SHARD, C], F32, kind="ExternalOutput")

    with tile.TileContext(nc) as tc:
        with tc.tile_pool(name="const", bufs=1) as constp, \
             tc.tile_pool(name="dram", bufs=1, space="DRAM") as dram:
            wqkv_sb = constp.tile([128, 8, 384], BF16)
            for cc in range(8):
                nc.sync.dma_start(
                    wqkv_sb[:, cc, :],
                    wqkv_d.ap().rearrange("(a p) n -> p a n", p=128)[:, cc, :])
            ident = constp.tile([128, 128], BF16)
            make_identity(nc, ident[:])

            # b0 uses one AllToAll; b1 is split four ways (one per qt pair)
            # so all but the last fire mid-phase. For b1 pair P, core
            # 4*(q%2)+j receives 128-token chunk j of q-tile q in {2P, 2P+1}.
            a2a_in0 = dram.tile([NCORES, 128, SHARD // 2], BF16,
                                name="a2a_in0", tag="a2a_in0")
            a2a_out0 = dram.tile([NCORES, 128, SHARD // 2], BF16,
                                 name="a2a_out0", tag="a2a_out0")
            a2a_in1 = [dram.tile([NCORES, 128, SHARD // 8], BF16,
                                 name=f"a2a_in1{h}", tag=f"a2a_in1{h}")
                       for h in range(4)]
            a2a_out1 = [dram.tile([NCORES, 128, SHARD // 8], BF16,
                                  name=f"a2a_out1{h}", tag=f"a2a_out1{h}")
                        for h in range(4)]

            with tc.tile_pool(name="batch", bufs=1) as bp, \
                 tc.tile_pool(name="work", bufs=3) as wp, \
                 tc.tile_pool(name="probsp", bufs=6) as pp, \
                 tc.tile_pool(name="pssc", bufs=3, space="PSUM") as pssc:

                qT = [bp.tile([128, N], BF16, name=f"qT{b}", tag=f"qT{b}") for b in range(B)]
                kT = [bp.tile([128, N], BF16, name=f"kT{b}", tag=f"kT{b}") for b in range(B)]
                vsb = [bp.tile([128, NB, 2, 65], BF16, name=f"v{b}", tag=f"v{b}")
                       for b in range(B)]
                # atn2[b][h]: col block pq holds q-chunks (2pq, 2pq+1) of head
                # half h transposed
                atn2 = [[bp.tile([128, N // 2], BF16, name=f"at{b}{h}",
                                 tag=f"at{b}{h}") for h in range(2)]
                        for b in range(B)]
                for b in range(B):
                    nc.vector.memset(vsb[b][:, :, :, 64:65], 1.0)

                # PE p-state warmup while the first hsT tiles stream in.
                # pssc tiles are [128, 1024] f32 = two banks; ring depth 3
                # gives the PE three kc of runahead so no per-iteration
                # dependency ever idles it (idle kicks the PE clock down).
                warm = pssc.tile([128, 1024], F32, name="warm", tag="pssc")
                for _ in range(144):
                    nc.tensor.matmul(warm[:, 0:128], lhsT=ident[:], rhs=ident[:],
                                     start=True, stop=True)

                # ---------------- Stage A: QKV + RMSNorm + RoPE ----------------
                def emit_A_load(b, mt, sp):
                    # one fused DMA for all 8 contraction chunks: each DMA
                    # trigger costs the SP sequencer ~585ns, so fewer, bigger
                    # transfers keep SP off the critical path
                    hs_t = sp.tile([128, 8, 256], BF16, name="hs", tag="hs")
                    nc.sync.dma_start(
                        hs_t[:],
                        hsT_d.ap()[:, b * N + mt * 256: b * N + (mt + 1) * 256]
                        .rearrange("(a p) n -> p a n", p=128))
                    trigC = sp.tile([128, 2, 256], BF16, name="trigC", tag="trigC")
                    trigS = sp.tile([128, 2, 256], BF16, name="trigS", tag="trigS")
                    for dst, dt_ in ((trigC, trigc_d), (trigS, trigs_d)):
                        nc.sync.dma_start(
                            dst[:], dt_.ap()[mt * 256:(mt + 1) * 256, :]
                            .rearrange("(s p) d -> p s d", p=128))
                    return hs_t, trigC, trigS

                def emit_A_mt(b, mt, sp, psT):
                    """One 256-token macro tile end to end: QKV (PE, one psum
                    tile with bank-disciplined chains), fused 512-wide psum
                    drains + squares (ACT), rinv via reciprocal+sqrt, 512-wide
                    RoPE (DVE 2x), PE transposes."""
                    hs_t, trigC, trigS = emit_A_load(b, mt, sp)
                    # one psum tile per mt: q+k chains @0:256/@256:512 (bank
                    # 0), v chains @512:640/@640:768 (bank 1); only the first
                    # chain per bank sets start=True.
                    ps_qkv = pssc.tile([128, 1024], F32, name="ps_qkv",
                                       tag="pssc")
                    for sub in range(2):
                        for cc in range(8):
                            lhs = hs_t[:, cc, sub * 128:(sub + 1) * 128]
                            nc.tensor.matmul(
                                ps_qkv[:, sub * 256:sub * 256 + 256], lhsT=lhs,
                                rhs=wqkv_sb[:, cc, 0:256],
                                start=(cc == 0 and sub == 0), stop=(cc == 7),
                                skip_group_check=(sub == 1))
                            nc.tensor.matmul(
                                ps_qkv[:, 512 + sub * 128:640 + sub * 128],
                                lhsT=lhs,
                                rhs=wqkv_sb[:, cc, 256:384],
                                start=(cc == 0 and sub == 0), stop=(cc == 7),
                                skip_group_check=(sub == 1))
                    qk_sb = wp.tile([128, 512], BF16, name="qk_sb",
                                    tag="qk_sb", bufs=4)
                    nc.scalar.copy(qk_sb[:], ps_qkv[:, 0:512])
                    nc.scalar.copy(
                        vsb[b][:, 2 * mt:2 * mt + 2, :, 0:64],
                        ps_qkv[:, 512:768].rearrange("p (s h d) -> p s h d",
                                                     s=2, h=2))
                    sq = wp.tile([128, 512], BF16, name="sq", tag="sq", bufs=4)
                    nc.scalar.square(sq[:], qk_sb[:])
                    # rinv = 8/sqrt(ssq): bit-trick seed + 1 Newton step, all
                    # on DVE (no cross-engine round trip)
                    ssq8 = wp.tile([128, 8], F32, name="ssq8", tag="ssq8", bufs=4)
                    nc.vector.tensor_reduce(
                        ssq8[:], sq[:].rearrange("p (a e) -> p a e", a=8),
                        axis=mybir.AxisListType.X, op=ALU.add)
                    yv = wp.tile([128, 8], F32, name="yv", tag="yv", bufs=4)
                    with nc.allow_low_precision(reason="rsqrt newton seed"):
                        nc.vector.tensor_scalar(
                            out=yv[:].bitcast(mybir.dt.int32),
                            in0=ssq8[:].bitcast(mybir.dt.int32),
                            scalar1=1, scalar2=None, op0=ALU.arith_shift_right)
                        nc.vector.tensor_scalar(
                            out=yv[:].bitcast(mybir.dt.int32),
                            in0=yv[:].bitcast(mybir.dt.int32),
                            scalar1=-1, scalar2=0x5F3759DF,
                            op0=ALU.mult, op1=ALU.add)
                    tn = wp.tile([128, 8], F32, name="tn", tag="tn", bufs=4)
                    nc.vector.tensor_mul(tn[:], yv[:], yv[:])
                    nc.vector.tensor_mul(tn[:], tn[:], ssq8[:])
                    nc.vector.tensor_scalar(out=tn[:], in0=tn[:],
                                            scalar1=-4.0, scalar2=12.0,
                                            op0=ALU.mult, op1=ALU.add)
                    nc.vector.tensor_mul(yv[:], yv[:], tn[:])
                    # RoPE on unnormalized values (bf16 2x, both subs at
                    # once); rinv applied last (it commutes with the rotation)
                    trigCf = trigC[:].rearrange("p s d -> p (s d)")
                    trigSf = trigS[:].rearrange("p s d -> p (s d)")
                    d_qk = wp.tile([128, 512], BF16, name="d_qk",
                                   tag="d_qk", bufs=3)
                    nc.vector.tensor_mul(d_qk[:], qk_sb[:], trigCf)
                    trot = wp.tile([128, 512], BF16, name="trot",
                                   tag="trot", bufs=3)
                    v4 = qk_sb[:].rearrange("p (a e) -> p a e", a=16)
                    s4 = trigSf.rearrange("p (a e) -> p a e", a=16)
                    t4 = trot[:].rearrange("p (a e) -> p a e", a=16)
                    nc.vector.tensor_mul(t4[:, 0:16:2, :], v4[:, 1:16:2, :],
                                         s4[:, 0:16:2, :])
                    nc.vector.tensor_mul(t4[:, 1:16:2, :], v4[:, 0:16:2, :],
                                         s4[:, 1:16:2, :])
                    rope = wp.tile([128, 512], BF16, name="rope",
                                   tag="rope", bufs=3)
                    nc.vector.tensor_add(rope[:], d_qk[:], trot[:])
                    d_bf = wp.tile([128, 512], BF16, name="d_bf",
                                   tag="d_bf", bufs=4)
                    nc.vector.tensor_tensor(
                        out=d_bf[:].rearrange("p (a e) -> p a e", a=8),
                        in0=rope[:].rearrange("p (a e) -> p a e", a=8),
                        in1=yv[:].unsqueeze(2).broadcast_to([128, 8, 64]),
                        op=ALU.mult)
                    def transposes(b=b, mt=mt, d_bf=d_bf):
                        for sub in range(2):
                            tt = mt * 2 + sub
                            for half, dst in ((0, qT[b]), (1, kT[b])):
                                # full-bank psum tile keeps later pools bank-
                                # aligned; only cols 0:128 are used
                                ps_t = psT.tile([128, 1024], BF16, name="ps_t",
                                                tag="pst")
                                nc.tensor.transpose(
                                    ps_t[:, 0:128],
                                    d_bf[:, (sub * 2 + half) * 128:
                                         (sub * 2 + half + 1) * 128],
                                    ident[:])
                                if sub == 1 and half == 1:
                                    nc.vector.tensor_copy(
                                        dst[:, tt * 128:(tt + 1) * 128],
                                        ps_t[:, 0:128])
                                else:
                                    nc.scalar.copy(
                                        dst[:, tt * 128:(tt + 1) * 128],
                                        ps_t[:, 0:128])
                    return transposes

                # ---------------- Stage B: attention --------------------------
                # Flipped AV: probs chunk [128kc, 128q] stationary, [v|1] moving.
                # at_ps[:, u, 0:65] (u = hh*4+j) accumulates [128q, 64d | denom].

                def stage_a2a(b, qt, srcblocks):
                    """Stage the transposed blocks of (b, qt) into the right
                    a2a DRAM buffer; fused APs keep the SP trigger count low
                    (each trigger costs ~585ns of SP sequencer time)."""
                    for hh in range(2):
                        for lp in range(2):
                            pq = 2 * qt + lp
                            src = atn2[b][hh][:, pq * 128:(pq + 1) * 128]
                            for rh in range(2):
                                blk = src[rh * 64:(rh + 1) * 64, :]
                                if b == 0:
                                    nc.sync.dma_start(
                                        a2a_in0[qt, hh * 64:(hh + 1) * 64,
                                                lp * 256 + rh * 128:
                                                lp * 256 + (rh + 1) * 128],
                                        blk)
                                else:
                                    # b1 a2a split 4 ways (one per qt pair):
                                    # core 4*(qt%2)+2*lp+rh gets this 128-tok
                                    # chunk of qt
                                    buf = a2a_in1[qt // 2]
                                    dest = 4 * (qt % 2) + 2 * lp + rh
                                    nc.sync.dma_start(
                                        buf[dest, hh * 64:(hh + 1) * 64, :],
                                        blk)

                # Cross-qt pipelined attention: the 2-deep AV delay queue and
                # the drain work span q-tile boundaries so the PE pipeline
                # never drains mid-phase.
                pipe = {"pend": [], "drains": []}

                def av_entry(e):
                    eb, eqt, eaps, pkc, ppr, last = e
                    for hh in range(2):
                        for j in range(4):
                            u = hh * 4 + j
                            nc.tensor.matmul(
                                eaps[:, u, 0:65],
                                lhsT=ppr[:, hh * QTILE + j * 128:
                                         hh * QTILE + (j + 1) * 128],
                                rhs=vsb[eb][:, pkc, hh, :],
                                start=(pkc == 0 and j == 0),
                                stop=last,
                                skip_group_check=(j != 0))
                    if not last:
                        return
                    rcp8 = wp.tile([128, 8], F32, name="rcp8", tag="rcp8", bufs=4)
                    nc.vector.reciprocal_approx_fast(
                        out=rcp8[:], in_=eaps[:, :, 64:65].rearrange(
                            "p u c -> p (u c)"))
                    dsbs = [None] * 4

                    def norm_act():
                        # ACT normalizes the even u's in an odd-kc slot (where
                        # ACT has no exp under the alternation)
                        for p in range(4):
                            dsbs[p] = wp.tile([128, 128], BF16, name="dsb",
                                              tag="dsb", bufs=8)
                        for u in (0, 2, 4, 6):
                            nc.scalar.activation(
                                dsbs[u // 2][:, (u % 2) * 64:(u % 2) * 64 + 64],
                                eaps[:, u, 0:64], ACTF.Copy,
                                bias=0.0, scale=rcp8[:, u:u + 1])

                    def norm_dve():
                        for u in (1, 3, 5, 7):
                            nc.vector.tensor_tensor(
                                out=dsbs[u // 2][:, (u % 2) * 64:
                                                 (u % 2) * 64 + 64],
                                in0=eaps[:, u, 0:64],
                                in1=rcp8[:, u:u + 1].broadcast_to([128, 64]),
                                op=ALU.mult)

                    def trans_stage():
                        for p in range(4):
                            hh, lp = divmod(p, 2)
                            pq = 2 * eqt + lp
                            nc.sync.dma_start_transpose(
                                atn2[eb][hh][:, pq * 128:(pq + 1) * 128],
                                dsbs[p][:])
                        stage_a2a(eb, eqt, dsbs)

                    pipe["drains"] += [norm_act, norm_dve, trans_stage]

                def emit_B(b, qt, psav, filler=None):
                    at_ps = psav.tile([128, 8, 128], F32, name="at_ps", tag="psav")

                    def scores(ps, kc, hh):
                        nc.tensor.matmul(
                            ps,
                            lhsT=kT[b][64 * hh:64 * (hh + 1),
                                       kc * 128:(kc + 1) * 128],
                            rhs=qT[b][64 * hh:64 * (hh + 1),
                                      qt * QTILE:(qt + 1) * QTILE],
                            start=True, stop=True)

                    for kc in range(KC):
                        ps_s = pssc.tile([128, 2 * QTILE], F32, name="ps_s",
                                         tag="pssc")
                        scores(ps_s[:, 0:QTILE], kc, 0)
                        scores(ps_s[:, QTILE:2 * QTILE], kc, 1)
                        pr = pp.tile([128, 2 * QTILE], BF16, name="pr", tag="pr",
                                     bufs=6)
                        # whole-tile exp alternating engines: halves each
                        # engine's per-instruction overhead; the depth-3 psum
                        # ring hides the longer latency, and the idle engine
                        # each kc absorbs the deferred drain pieces
                        if kc % 2 == 0:
                            nc.scalar.activation(pr[:], ps_s[:], ACTF.Exp,
                                                 bias=0.0, scale=0.125)
                        else:
                            with nc.allow_low_precision(reason="schraudolph exp"):
                                nc.vector.tensor_scalar(
                                    out=pr[:].bitcast(U16),
                                    in0=ps_s[:],
                                    scalar1=float(SCHRAUD_A),
                                    scalar2=float(SCHRAUD_B),
                                    op0=ALU.mult, op1=ALU.add)
                        # drain pieces for kc>=2 go ahead of the AV block so
                        # the normalizes that free the previous q-tile's
                        # accumulator run before the first AV that reuses it
                        if pipe["drains"] and kc in (2, 3):
                            pipe["drains"].pop(0)()
                        if len(pipe["pend"]) == 2:
                            av_entry(pipe["pend"].pop(0))
                        if pipe["drains"] and kc == 1:
                            pipe["drains"].pop(0)()
                        if filler is not None:
                            filler(kc)
                        pipe["pend"].append(
                            (b, qt, at_ps, kc, pr, kc == KC - 1))

                def flush_pipe():
                    while pipe["pend"]:
                        av_entry(pipe["pend"].pop(0))
                    while pipe["drains"]:
                        pipe["drains"].pop(0)()

                # ---- phase 1: ALL stage A (both batches) --------------------
                # Transposes deferred one macro-tile so the PE queue never
                # waits on the DVE chain's tail.
                with tc.tile_pool(name="stream", bufs=6) as sp:
                    with tc.tile_pool(name="psT", bufs=2, space="PSUM") as psT:
                        pend_t = None
                        for b in range(B):
                            for mt in range(NMACRO):
                                nt = emit_A_mt(b, mt, sp, psT)
                                if pend_t is not None:
                                    pend_t()
                                pend_t = nt
                        pend_t()

                # ---- phase 2: batch-0 attention -----------------------------
                with tc.tile_pool(name="psav", bufs=1, space="PSUM") as psav:
                    for qt in range(NQT):
                        emit_B(0, qt, psav)
                    flush_pipe()

                    nc.gpsimd.collective_compute(
                        "AllToAll", ALU.bypass,
                        ins=[a2a_in0[:].opt()], outs=[a2a_out0[:].opt()],
                        replica_groups=[list(range(NCORES))])

                    # ---- phase 3: batch-1 attention + batch-0 out proj ------
                    with tc.tile_pool(name="cstage", bufs=1) as cp, \
                         tc.tile_pool(name="cwork", bufs=2) as cw:
                        wout_sb = cp.tile([128, 8, C], BF16)
                        nc.sync.dma_start(
                            wout_sb[:],
                            wout_d.ap().rearrange("(a p) n -> p a n", p=128))
                        atf = cp.tile([128, 8, SHARD], BF16)
                        nc.sync.dma_start(atf[:, :, 0:512],
                                          a2a_out0[:].transpose([1, 0, 2]))

                        osts = {}

                        def emit_C(ttk, half, drain_eng):
                            if half == 0:
                                osts[ttk] = cw.tile([128, C], F32,
                                                    name="ostage", tag="ostage")
                            ost = osts[ttk]
                            ps_o = pssc.tile([128, 1024], F32, name="ps_o",
                                             tag="pssc")
                            for cc in range(8):
                                lhs = atf[:, cc, ttk * 128:(ttk + 1) * 128]
                                nc.tensor.matmul(
                                    ps_o[:, 0:256], lhsT=lhs,
                                    rhs=wout_sb[:, cc,
                                                half * 512:half * 512 + 256],
                                    start=(cc == 0), stop=(cc == 7))
                                nc.tensor.matmul(
                                    ps_o[:, 512:768], lhsT=lhs,
                                    rhs=wout_sb[:, cc, half * 512 + 256:
                                                half * 512 + 512],
                                    start=(cc == 0), stop=(cc == 7))
                            d0 = ost[:, half * 512:half * 512 + 256]
                            d1 = ost[:, half * 512 + 256:half * 512 + 512]
                            if drain_eng == "act":
                                nc.scalar.copy(d0, ps_o[:, 0:256])
                                nc.scalar.copy(d1, ps_o[:, 512:768])
                            else:
                                nc.vector.tensor_copy(d0, ps_o[:, 0:256])
                                nc.vector.tensor_copy(d1, ps_o[:, 512:768])
                            nc.sync.dma_start(
                                out_d.ap()[ttk * 128:(ttk + 1) * 128,
                                           half * 512:(half + 1) * 512],
                                ost[:, half * 512:(half + 1) * 512])

                        cunits = [(t, h) for t in range(4) for h in range(2)]

                        for qt in range(NQT):
                            def extra(kc, qt=qt):
                                if kc == 16 and qt >= 2 and cunits:
                                    emit_C(*cunits.pop(0), drain_eng="vec")
                            emit_B(1, qt, psav, filler=extra)
                            if qt >= 2 and qt % 2 == 0:
                                # qt pair P=(qt-2)//2 fully staged (its drains
                                # completed in earlier q-tiles' slots)
                                P = (qt - 2) // 2
                                nc.gpsimd.collective_compute(
                                    "AllToAll", ALU.bypass,
                                    ins=[a2a_in1[P][:].opt()],
                                    outs=[a2a_out1[P][:].opt()],
                                    replica_groups=[list(range(NCORES))])
                                nc.sync.dma_start(
                                    atf[:, :, 512 + 128 * P:640 + 128 * P],
                                    a2a_out1[P][:].transpose([1, 0, 2]))
                        flush_pipe()

                        # last qt pair: only 0.25MB left in flight at the end
                        nc.gpsimd.collective_compute(
                            "AllToAll", ALU.bypass,
                            ins=[a2a_in1[3][:].opt()],
                            outs=[a2a_out1[3][:].opt()],
                            replica_groups=[list(range(NCORES))])
                        nc.sync.dma_start(
                            atf[:, :, 896:1024],
                            a2a_out1[3][:].transpose([1, 0, 2]))

                        while cunits:  # leftover b0 units overlap the a2a
                            emit_C(*cunits.pop(0), drain_eng="act")
                        # ttk 4-5 landed long ago; 6-7 overlap the last a2a
                        for ttk in (4, 5, 6, 7):
                            for half in range(2):
                                emit_C(ttk, half, drain_eng="act")

    nc.compile()
    return nc


def _fold_sin(sin, g):
    out = np.empty_like(sin)
    out[:, :32] = -sin[:, :32] * g[32:]
    out[:, 32:] = sin[:, 32:] * g[:32]
    return out


def kernel(hidden_states, cos, sin, Wqkv, Wout, gq, gk):
    global _LAST_RESULT
    _install_profile_shim()

    hidden_states = np.asarray(hidden_states, dtype=np.float32)
    cos = np.asarray(cos, dtype=np.float32)
    sin = np.asarray(sin, dtype=np.float32)
    Wqkv = np.asarray(Wqkv, dtype=np.float32)
    Wout = np.asarray(Wout, dtype=np.float32)
    gq = np.asarray(gq, dtype=np.float32)
    gk = np.asarray(gk, dtype=np.float32)

    if "nc" not in _CACHE:
        _CACHE["nc"] = _build_graph()
    nc = _CACHE["nc"]

    hsT = np.ascontiguousarray(hidden_states.reshape(TOK, C).T).astype(ml_dtypes.bfloat16)
    cosq = cos * gq[None, :]
    sinq = _fold_sin(sin, gq)
    cosk = cos * gk[None, :]
    sink = _fold_sin(sin, gk)
    trigc = np.concatenate([cosq, cosq, cosk, cosk], axis=1).astype(ml_dtypes.bfloat16)
    trigs = np.concatenate([sinq, sinq, sink, sink], axis=1).astype(ml_dtypes.bfloat16)
    wout_bf = Wout.astype(ml_dtypes.bfloat16)

    in_maps = []
    for c in range(NCORES):
        wq = Wqkv[:, c * 128:(c + 1) * 128]
        wk = Wqkv[:, C + c * 128:C + (c + 1) * 128]
        wv = Wqkv[:, 2 * C + c * 128:2 * C + (c + 1) * 128]
        wqkv_loc = np.ascontiguousarray(
            np.concatenate([wq, wk, wv], axis=1)).astype(ml_dtypes.bfloat16)
        in_maps.append({
            "hsT": hsT, "wqkv": wqkv_loc, "trigc": trigc, "trigs": trigs,
            "wout": wout_bf,
        })

    trace = bool(os.environ.get("BASS_TRACE"))
    res = run_bass_kernel_spmd(nc, in_maps, core_ids=list(range(NCORES)), trace=trace)
    _LAST_RESULT = res

    full = np.empty((B, N, C), dtype=np.float32)
    for c in range(NCORES):
        o = res.results[c]["out"]
        # batch 0: plain token sharding
        full[0, c * 512:(c + 1) * 512, :] = o[0:512]
        # batch 1: core c owns 128-token chunk c%4 of q-tile 2P + c//4 for
        # each qt pair P
        for P in range(4):
            qt = 2 * P + c // 4
            j = c % 4
            full[1, qt * 512 + j * 128: qt * 512 + (j + 1) * 128, :] = \
                o[512 + 128 * P: 640 + 128 * P]
    return full
